# revision 9
# baseline (speedup 1.0000x reference)
"""CrossAttention kernel for 8x TRN2 NeuronCores (Bass/Tile), v2.

Reference computation (per batch b of 16, heads h=8, n=1024, d_model=512, dh=64):
    q = x @ Wq.T, k = x @ Wk.T, v = x @ Wv.T          (per-head slices)
    sim = q k^T * scale + rel_bias[h]
    attn = softmax(sim, axis=-1)
    out = (attn @ v) re-assembled over heads, then @ Wo.T + bo

Sharding: data-parallel over batch, 2 batches per core x 8 cores.

Design notes (engine balance targets, per-core):
  - ACT is the floor: 128 exp instructions over [128,1024] PSUM->SBUF bf16
    (~133us).  Everything else is arranged to hide under it.
  - sim computed TRANSPOSED per (h, bi, jt): simT[j, i] on PSUM; softmax
    max-subtraction skipped (logits O(1)); rel_bias folded as
    exp(sim)*exp(bias^T) (bf16 from host), multiplied in-place on DVE
    (2x bf16 mode).
  - attn@v FLIPPED: out[i, d] with M=128 (i on PSUM partitions) accumulated
    over jt.  VA carries an appended ones column so out[:, 64] is the softmax
    denominator l[i].  Halves PE rows vs the [d, i] orientation.
  - normalization: one strided reciprocal + ONE stride-0 broadcast DVE
    multiply per (h, bi) into AOn[i, it, h*64+d].
  - AO^T via XBAR dma_start_transpose (bf16); output projection consumes AOT
    directly; bo added on HOST.
  - attnv/norm run one stream BEHIND sims (software pipeline) so the ACT exp
    stream never waits on PE stream turnaround.
  - DMAs are consolidated into big-tile transfers (HWDGE fixed cost is
    ~625ns per DMA, serialized).
  - P0 psum drains on ACT (idle before attention), P1 on DVE (slack during
    attention).
"""

import numpy as np
import ml_dtypes

HEADS = 8
DH = 64
B = 16
N = 1024
D = 512  # d_model = inner
SCALE = DH ** -0.5
NCORES = 8
BPC = B // NCORES  # batches per core

NT = N // 128   # 8 n-tiles
KP = D // 128   # 4 k-tiles of d_model
EPC = D // 128  # 4 e-slices of inner


def build_nc(et_bufs=2, eb_bufs=2, xt_bufs=2, fo_bufs=2):
    import concourse.mybir as mybir
    import concourse.tile as tile
    from concourse import bacc

    f32 = mybir.dt.float32
    f16 = mybir.dt.float16
    f32r = mybir.dt.float32r
    bf16 = mybir.dt.bfloat16
    Exp = mybir.ActivationFunctionType.Exp
    Copy = mybir.ActivationFunctionType.Copy

    nc = bacc.Bacc(None, target_bir_lowering=False)

    xT_d = nc.dram_tensor("xT", [BPC, D, N], bf16, kind="ExternalInput")
    wq_d = nc.dram_tensor("WqT", [D, D], bf16, kind="ExternalInput")   # pre-scaled
    wk_d = nc.dram_tensor("WkT", [D, D], bf16, kind="ExternalInput")
    wv_d = nc.dram_tensor("WvT", [D, D], bf16, kind="ExternalInput")
    wo_d = nc.dram_tensor("WoT", [D, D], bf16, kind="ExternalInput")
    eb_d = nc.dram_tensor("expBT", [HEADS, N, N], bf16, kind="ExternalInput")
    out_d = nc.dram_tensor("out", [BPC, N, D], f16, kind="ExternalOutput")

    with tile.TileContext(nc) as tc:
        with (
            tc.tile_pool(name="pers", bufs=1) as pers,
            tc.tile_pool(name="rec", bufs=2) as recp,
            tc.tile_pool(name="fo", bufs=fo_bufs) as fop,
        ):
            # ---- persistent tiles
            QT = [[pers.tile([128, N], bf16, tag=f"qt{bi}_{ip}", name=f"qt{bi}_{ip}")
                   for ip in range(KP)] for bi in range(BPC)]
            KT = [[pers.tile([128, N], bf16, tag=f"kt{bi}_{ip}", name=f"kt{bi}_{ip}")
                   for ip in range(KP)] for bi in range(BPC)]
            # V with ones cols: [j(128), jt, h*(DH+1)+d]
            VA = [pers.tile([128, NT, HEADS * (DH + 1)], bf16, tag=f"va{bi}", name=f"va{bi}")
                  for bi in range(BPC)]
            # normalized attn out, [i(128), it, h*DH+d]
            AOn = [pers.tile([128, NT, D], bf16, tag=f"aon{bi}", name=f"aon{bi}")
                   for bi in range(BPC)]
            # transposed: [e(128), ep, i]
            AOT = [pers.tile([128, EPC, N], bf16, tag=f"aot{bi}", name=f"aot{bi}")
                   for bi in range(BPC)]
            wo_s = pers.tile([128, EPC, D], bf16, tag="wo", name="wo")

            # stack-allocated pools: mid-kernel-released pools must be on top
            etp = tc.alloc_tile_pool(name="et", bufs=et_bufs)
            ebp = tc.alloc_tile_pool(name="eb", bufs=eb_bufs)
            wqkv = tc.alloc_tile_pool(name="wqkv", bufs=1)
            xtp = tc.alloc_tile_pool(name="xt", bufs=xt_bufs)
            ps_sim = tc.alloc_tile_pool(name="ps_sim", bufs=2, space="PSUM")
            ps_o = tc.alloc_tile_pool(name="ps_o", bufs=1, space="PSUM")
            ps_p = tc.alloc_tile_pool(name="ps_p", bufs=2, space="PSUM")

            wq_s = wqkv.tile([128, KP, D], bf16, tag="wq", name="wq")
            wk_s = wqkv.tile([128, KP, D], bf16, tag="wk", name="wk")
            wv_s = wqkv.tile([128, KP, D], bf16, tag="wv", name="wv")
            nc.sync.dma_start(
                out=wq_s[:], in_=wq_d[:].rearrange("(a p) e -> p a e", p=128))
            nc.sync.dma_start(
                out=wk_s[:], in_=wk_d[:].rearrange("(a p) e -> p a e", p=128))

            def load_xt(bi):
                xt = xtp.tile([128, KP, N], bf16, tag="xt", name="xt")
                nc.sync.dma_start(
                    out=xt[:], in_=xT_d[bi].rearrange("(a p) n -> p a n", p=128))
                return xt

            xt0 = load_xt(0)

            eb_t = {}

            def load_eb(h):
                t = ebp.tile([128, NT, N], bf16, tag="eb", name="eb")
                nc.sync.dma_start(
                    out=t[:], in_=eb_d[h].rearrange("(a p) i -> p a i", p=128))
                eb_t[h] = t

            load_eb(0)
            nc.sync.dma_start(
                out=wv_s[:], in_=wv_d[:].rearrange("(a p) e -> p a e", p=128))
            nc.sync.dma_start(
                out=wo_s[:], in_=wo_d[:].rearrange("(a p) e -> p a e", p=128))

            # ones columns of VA (written once; V copies fill the rest)
            for bi in range(BPC):
                ones_view = VA[bi][:].rearrange(
                    "p a (h c) -> p (a h) c", c=DH + 1)[:, :, DH:DH + 1]
                nc.gpsimd.memset(ones_view, 1.0)

            def proj_qk_ip(bi, xt, W_s, DST, ip):
                """One ip-column of a QT/KT projection (8 matmuls + 2 copies)."""
                for nh in range(2):
                    pt = ps_p.tile([128, 512], f32, tag="pp", name="pp")
                    for kp in range(KP):
                        nc.tensor.matmul(
                            pt[:],
                            W_s[:, kp, ip * 128:(ip + 1) * 128],
                            xt[:, kp, nh * 512:(nh + 1) * 512],
                            start=(kp == 0), stop=(kp == KP - 1),
                        )
                    nc.vector.tensor_copy(
                        out=DST[ip][:, nh * 512:(nh + 1) * 512], in_=pt[:])

            def proj_v_half(bi, xt, half):
                """V natural [n, e] -> VA[bi] bf16, 4 n-tiles at a time."""
                for nt in range(half * 4, half * 4 + 4):
                    pt = ps_p.tile([128, 512], f32, tag="pp", name="pp")
                    for kp in range(KP):
                        nc.tensor.matmul(
                            pt[:],
                            xt[:, kp, nt * 128:(nt + 1) * 128],
                            wv_s[:, kp, :],
                            start=(kp == 0), stop=(kp == KP - 1),
                        )
                    dst = VA[bi][:, nt, :].rearrange("p (h c) -> p h c", c=DH + 1)[:, :, 0:DH]
                    src = pt[:].rearrange("p (h c) -> p h c", c=DH)
                    nc.vector.tensor_copy(out=dst, in_=src)

            xt1 = load_xt(1)
            xts = [xt0, xt1]

            def qk_chunk(bi, ip):
                proj_qk_ip(bi, xts[bi], wq_s, QT[bi], ip)
                proj_qk_ip(bi, xts[bi], wk_s, KT[bi], ip)

            # ---- head: only the ip0 projections of batch 0 gate the first stream
            qk_chunk(0, 0)

            # deferred projection chunks: (iter, slot) -> [closure]
            # slot 0 = before sims, 1 = after sims, 2 = after attnv
            deferred = {
                (0, 1): [lambda: proj_v_half(0, xt0, 0)],
                (0, 2): [lambda: qk_chunk(1, 0)],
                (1, 1): [lambda: proj_v_half(0, xt0, 1)],
                (1, 2): [lambda: proj_v_half(1, xt1, 0)],
                (2, 1): [lambda: proj_v_half(1, xt1, 1)],
                (3, 1): [lambda: proj_qk_ip(0, xt0, wq_s, QT[0], 1)],
                (3, 2): [lambda: proj_qk_ip(0, xt0, wk_s, KT[0], 1)],
                (4, 1): [lambda: proj_qk_ip(1, xt1, wq_s, QT[1], 1)],
                (4, 2): [lambda: proj_qk_ip(1, xt1, wk_s, KT[1], 1)],
                (7, 1): [lambda: proj_qk_ip(0, xt0, wq_s, QT[0], 2)],
                (7, 2): [lambda: proj_qk_ip(0, xt0, wk_s, KT[0], 2)],
                (8, 1): [lambda: proj_qk_ip(1, xt1, wq_s, QT[1], 2)],
                (8, 2): [lambda: proj_qk_ip(1, xt1, wk_s, KT[1], 2)],
                (11, 1): [lambda: proj_qk_ip(0, xt0, wq_s, QT[0], 3)],
                (11, 2): [lambda: proj_qk_ip(0, xt0, wk_s, KT[0], 3)],
                (12, 1): [lambda: proj_qk_ip(1, xt1, wq_s, QT[1], 3)],
                (12, 2): [lambda: proj_qk_ip(1, xt1, wk_s, KT[1], 3)],
            }

            def run_deferred(k, slot):
                for fn in deferred.pop((k, slot), ()):
                    fn()

            def sims(h, bi, hooks=None):
                """sim + exp + bias-mul for one stream; returns et."""
                ip = h // 2
                po = (h % 2) * DH
                et = etp.tile([128, NT, N], bf16, tag="et", name="et")
                for jt in range(NT):
                    sp = ps_sim.tile([128, N], f32, tag="sim", name="sim")
                    for ihh in range(2):
                        nc.tensor.matmul(
                            sp[:, ihh * 512:(ihh + 1) * 512],
                            KT[bi][ip][po:po + DH, jt * 128:(jt + 1) * 128],
                            QT[bi][ip][po:po + DH, ihh * 512:(ihh + 1) * 512],
                            start=True, stop=True,
                        )
                    nc.scalar.activation(et[:, jt, :], sp[:], Exp)
                    nc.vector.tensor_mul(
                        out=et[:, jt, :], in0=et[:, jt, :], in1=eb_t[h][:, jt, :])
                    if hooks:
                        for fn in hooks.pop(jt, ()):
                            fn()
                return et

            def attnv(h, bi, et):
                """attn@v (flipped; each it-group contiguous) + normalization."""
                o = ps_o.tile([128, NT, 128], f32, tag="o", name="o")
                for it in range(NT):
                    for jt in range(NT):
                        nc.tensor.matmul(
                            o[:, it, 0:DH + 1],
                            et[:, jt, it * 128:(it + 1) * 128],
                            VA[bi][:, jt, h * (DH + 1):(h + 1) * (DH + 1)],
                            start=(jt == 0), stop=(jt == NT - 1),
                        )
                rec = recp.tile([128, NT], f32, tag="rec", name="rec")
                nc.vector.reciprocal_approx_fast(out=rec[:], in_=o[:, :, DH])
                nc.vector.tensor_mul(
                    out=AOn[bi][:, :, h * DH:(h + 1) * DH],
                    in0=o[:, :, 0:DH],
                    in1=rec[:, :, None].broadcast_to([128, NT, DH]),
                )

            def trans_final(bi, nts):
                """Interleaved AO^T transpose + output projection, per n-tile."""
                for nt in nts:
                    nc.sync.dma_start_transpose(
                        out=AOT[bi][:, :, nt * 128:(nt + 1) * 128],
                        in_=AOn[bi][:, nt, :],
                    )
                    if nt % 4 == 0:
                        fo = fop.tile([128, 4, D], f16, tag="fo", name="fo")
                        fo_t[bi, nt // 4] = fo
                    fo = fo_t[bi, nt // 4]
                    fp = ps_f.tile([128, D], f32, tag="fp", name="fp")
                    for ep in range(EPC):
                        nc.tensor.matmul(
                            fp[:],
                            AOT[bi][:, ep, nt * 128:(nt + 1) * 128],
                            wo_s[:, ep, :],
                            start=(ep == 0), stop=(ep == EPC - 1),
                        )
                    nc.vector.tensor_copy(out=fo[:, nt % 4, :], in_=fp[:])
                    if nt % 4 == 3:
                        half = nt // 4
                        dst = out_d[bi, half * 512:(half + 1) * 512, :]
                        nc.sync.dma_start(
                            out=dst.rearrange("(a p) c -> p a c", p=128), in_=fo[:])

            fo_t = {}

            # ---- attention streams, software-pipelined (attnv one stream back)
            ps_f = None
            streams = [(h, bi) for h in range(HEADS) for bi in range(BPC)]
            prev = None
            for k, (h, bi) in enumerate(streams):
                if bi == 0 and h + 1 < HEADS:
                    load_eb(h + 1)  # prefetch next head's bias
                run_deferred(k, 0)
                hooks = {2: deferred.pop((k, 1), []), 5: deferred.pop((k, 2), [])}
                et = sims(h, bi, hooks)
                if k == 13:
                    assert not deferred, f"unemitted chunks: {list(deferred)}"
                    ps_p.release()
                    xtp.release()
                    wqkv.release()
                    ps_f = tc.alloc_tile_pool(name="ps_f", bufs=2, space="PSUM")
                if prev is not None:
                    attnv(*prev)
                    if prev[:2] == (HEADS - 1, 0):
                        trans_final(0, range(NT))
                prev = (h, bi, et)
            attnv(*prev)
            trans_final(1, range(NT))

            ps_f.release()
            ps_o.release()
            ps_sim.release()
            ebp.release()
            etp.release()

    nc.compile()
    return nc


def prep_inputs(x, Wq, Wk, Wv, rel_bias, Wo, bo, ncores=NCORES, bpc=BPC):
    """Host-side sharding/layout prep. Returns in_maps (one dict per core)."""
    x = np.ascontiguousarray(x, dtype=np.float32)
    xT = np.ascontiguousarray(x.transpose(0, 2, 1)).astype(ml_dtypes.bfloat16)  # [B, D, n]
    bf = ml_dtypes.bfloat16
    WqT = np.ascontiguousarray(Wq.T * np.float32(SCALE)).astype(bf)
    WkT = np.ascontiguousarray(Wk.T, dtype=np.float32).astype(bf)
    WvT = np.ascontiguousarray(Wv.T, dtype=np.float32).astype(bf)
    WoT = np.ascontiguousarray(Wo.T, dtype=np.float32).astype(bf)
    expBT = np.ascontiguousarray(
        np.exp(rel_bias.astype(np.float32).transpose(0, 2, 1))
    ).astype(bf)                                            # [H, n(j), n(i)]
    in_maps = []
    for c in range(ncores):
        in_maps.append({
            "xT": np.ascontiguousarray(xT[c * bpc:(c + 1) * bpc]),
            "WqT": WqT, "WkT": WkT, "WvT": WvT, "WoT": WoT,
            "expBT": expBT,
        })
    return in_maps


_CACHE = {}


def kernel(x, Wq, Wk, Wv, rel_bias, Wo, bo):
    from concourse.bass_utils import run_bass_kernel_spmd

    if "nc" not in _CACHE:
        _CACHE["nc"] = build_nc()
    nc = _CACHE["nc"]
    in_maps = prep_inputs(x, Wq, Wk, Wv, rel_bias, Wo, bo)
    res = run_bass_kernel_spmd(nc, in_maps, core_ids=list(range(NCORES)))
    out = np.concatenate(
        [res.results[c]["out"].astype(np.float32) for c in range(NCORES)], axis=0)
    out = out + np.asarray(bo, dtype=np.float32)[None, None, :]
    return np.ascontiguousarray(out, dtype=np.float32)
